# revision 24
# baseline (speedup 1.0000x reference)
"""Bahdanau attention Trainium2 kernel.

reference:
    wq = query @ Wa_w.T + Wa_b                       # [B, M]
    uk = einsum('bse,me->bsm', keys, Ua_w) + Ua_b    # [B, S, M]
    e = tanh(wq[:, None, :] + uk)                    # [B, S, M]
    scores = einsum('bsm,om->bos', e, Va_w) + Va_b   # [B, 1, S]
    scores = where(mask, -inf, scores)
    attn = softmax(scores, axis=-1)
    context = einsum('bos,bse->boe', attn, keys)     # [B, 1, 2d]

Sharding: data-parallel over batch, 4 batches per core on 8 cores.
Matmul data path in bf16 (1 cycle/row on the PE; fp32r streams at ~2
cycles/row on this HW), accumulation in fp32 PSUM.
"""

import functools

import numpy as np
import ml_dtypes

import concourse.bacc as bacc
import concourse.tile as tile
from concourse import mybir
from concourse.bass_utils import run_bass_kernel_spmd

N_CORES = 8
B, S, D = 32, 2048, 512
E = 2 * D            # 1024 (key embedding dim)
M = D                # 512  (model dim)
BL = B // N_CORES    # 4 batches per core
SC = S // 512        # 4 s-chunks of 512
ST = S // 128        # 16 s-tiles of 128
ET = E // 128        # 8 e-tiles of 128
MT = M // 128        # 4 m-tiles of 128
NAT_G = 2            # s-tiles per natural-keys DMA
NEG = np.float32(-1e30)
BF16NP = ml_dtypes.bfloat16

F32 = mybir.dt.float32
BF16 = mybir.dt.bfloat16
AF = mybir.ActivationFunctionType


def build():
    nc = bacc.Bacc(None, target_bir_lowering=False)

    keys_nat = nc.dram_tensor("keys_nat", [BL, ST, 128, E], BF16, kind="ExternalInput")
    keys_t = nc.dram_tensor("keys_t", [BL, SC, ET, 128, 512], BF16, kind="ExternalInput")
    ua_t = nc.dram_tensor("ua_t", [ET, 128, M], BF16, kind="ExternalInput")
    vat = nc.dram_tensor("vat", [128, 8], BF16, kind="ExternalInput")
    wq_b = nc.dram_tensor("wq_b", [128, BL * MT + 1], F32, kind="ExternalInput")
    maskb = nc.dram_tensor("maskb", [BL, S], F32, kind="ExternalInput")
    out_ctx = nc.dram_tensor("out_ctx", [BL, E], F32, kind="ExternalOutput")

    with tile.TileContext(nc) as tc:
        with (
            tc.tile_pool(name="const", bufs=1) as constp,
            tc.tile_pool(name="kt", bufs=4) as ktp,
            tc.tile_pool(name="nat", bufs=8) as natp,
            tc.tile_pool(name="ep", bufs=6) as ep,
            tc.tile_pool(name="sm", bufs=2) as smp,
            tc.tile_pool(name="ukps", bufs=2, space="PSUM") as ukps,
            tc.tile_pool(name="scps", bufs=2, space="PSUM") as scps,
            tc.tile_pool(name="sctps", bufs=1, space="PSUM") as sctps,
            tc.tile_pool(name="ctxps", bufs=1, space="PSUM") as ctxps,
        ):
            ua_sb = constp.tile([128, ET, M], BF16)
            nc.scalar.dma_start(ua_sb[:], ua_t[:].rearrange("a p m -> p a m"))
            vat_sb = constp.tile([128, 8], BF16)
            nc.gpsimd.dma_start(vat_sb[:], vat[:])
            wq_sb = constp.tile([128, BL * MT + 1], F32)
            nc.gpsimd.dma_start(wq_sb[:], wq_b[:])

            def phase_a_chunk(b, sc_i, state):
                """emit one s-chunk of the scores computation"""
                if sc_i == 0:
                    state["scores_row"] = smp.tile([1, S], F32, tag="scores", name="scores_row")
                scores_row = state["scores_row"]
                kt_sb = ktp.tile([128, ET, 512], BF16, tag="kt")
                nc.sync.dma_start(kt_sb[:], keys_t[b, sc_i].rearrange("a p j -> p a j"))
                mask_sb = smp.tile([1, 512], F32, tag="mask", bufs=3)
                nc.gpsimd.dma_start(mask_sb[:], maskb[b : b + 1, sc_i * 512 : (sc_i + 1) * 512])
                ps_sc = scps.tile([1, 512], F32, tag="ps_sc")
                e_tiles = []
                for mt_i in range(MT):
                    ps_uk = ukps.tile([128, 512], F32, tag="ps_uk")
                    for et_i in range(ET):
                        nc.tensor.matmul(
                            ps_uk[:],
                            ua_sb[:, et_i, mt_i * 128 : (mt_i + 1) * 128],
                            kt_sb[:, et_i, :],
                            start=(et_i == 0),
                            stop=(et_i == ET - 1),
                        )
                    e_sb = ep.tile([128, 512], BF16, tag="e")
                    nc.scalar.activation(
                        e_sb[:], ps_uk[:], AF.Tanh,
                        bias=wq_sb[:, b * MT + mt_i : b * MT + mt_i + 1],
                    )
                    e_tiles.append(e_sb)
                for mt_i in range(MT):
                    nc.tensor.matmul(
                        ps_sc[:],
                        vat_sb[:, mt_i : mt_i + 1],
                        e_tiles[mt_i][:],
                        start=(mt_i == 0),
                        stop=(mt_i == MT - 1),
                    )
                # scores_row = psum + mask(-1e30 or Va_b)
                nc.vector.tensor_add(
                    scores_row[0:1, sc_i * 512 : (sc_i + 1) * 512], ps_sc[:], mask_sb[:]
                )

            def phase_a_fin(b, state):
                """exp + denom + PE transpose of scores"""
                scores_row = state["scores_row"]
                exp_row = smp.tile([1, S], F32, tag="exp_row")
                denom = smp.tile([1, 1], F32, tag="denom")
                nc.scalar.activation(exp_row[:], scores_row[:], AF.Exp, accum_out=denom[:])
                recip = smp.tile([1, 1], F32, tag="recip")
                nc.vector.reciprocal(recip[:], denom[:])
                # Cross-partition SBUF->SBUF scatter DMA corrupts data on HW, so
                # transpose scores on the PE instead: column j of ps_sct is
                # scores[128j:128j+128].T @ [[1.0]]  (K=1, M=128, N=1 fp32 matmul;
                # fp32r/bf16 matmuls forbid odd N, plain fp32 allows it).
                ps_sct = sctps.tile([128, ST], F32, tag="ps_sct")
                for j in range(ST):
                    nc.tensor.matmul(
                        ps_sct[:, j : j + 1],
                        scores_row[0:1, j * 128 : (j + 1) * 128],
                        wq_sb[0:1, BL * MT : BL * MT + 1],
                        start=True,
                        stop=True,
                    )
                exp_t = smp.tile([128, ST], BF16, tag="exp_t")
                nc.scalar.activation(exp_t[:], ps_sct[:], AF.Exp)
                state["exp_t"] = exp_t
                state["recip"] = recip

            def phase_c_q(b, q, state):
                """emit one natural-keys group (NAT_G s-tiles) of the context matmul"""
                exp_t = state["exp_t"]
                if q == 0:
                    state["ps0"] = ctxps.tile([1, 512], F32, tag="ps_ctx0", name="ps0")
                    state["ps1"] = ctxps.tile([1, 512], F32, tag="ps_ctx1", name="ps1")
                ps0, ps1 = state["ps0"], state["ps1"]
                nat_sb = natp.tile([128, NAT_G, E], BF16, tag="nat")
                nc.scalar.dma_start(
                    nat_sb[:], keys_nat[b, q * NAT_G : (q + 1) * NAT_G].rearrange("t p e -> p t e")
                )
                for r in range(NAT_G):
                    st_i = q * NAT_G + r
                    nc.tensor.matmul(
                        ps0[:], exp_t[:, st_i : st_i + 1], nat_sb[:, r, 0:512],
                        start=(st_i == 0), stop=(st_i == ST - 1),
                    )
                    nc.tensor.matmul(
                        ps1[:], exp_t[:, st_i : st_i + 1], nat_sb[:, r, 512:1024],
                        start=(st_i == 0), stop=(st_i == ST - 1),
                    )

            def phase_c_fin(b, state):
                ctx_sb = smp.tile([1, E], F32, tag="ctx")
                nc.scalar.activation(ctx_sb[0:1, 0:512], state["ps0"][:], AF.Copy, scale=state["recip"][:])
                nc.scalar.activation(ctx_sb[0:1, 512:1024], state["ps1"][:], AF.Copy, scale=state["recip"][:])
                nc.sync.dma_start(out_ctx[b : b + 1, :], ctx_sb[:])

            # Software pipeline: interleave phase C of batch b with phase A of
            # batch b+1 at chunk granularity so PE and both DMA rings stay busy.
            NQ = ST // NAT_G
            states = [dict() for _ in range(BL)]
            for sc_i in range(SC):
                phase_a_chunk(0, sc_i, states[0])
            phase_a_fin(0, states[0])
            for b in range(1, BL):
                qn = 0
                for sc_i in range(SC):
                    phase_a_chunk(b, sc_i, states[b])
                    while qn < (sc_i + 1) * NQ // SC:
                        phase_c_q(b - 1, qn, states[b - 1])
                        qn += 1
                phase_a_fin(b, states[b])
                phase_c_fin(b - 1, states[b - 1])
            for qn in range(NQ):
                phase_c_q(BL - 1, qn, states[BL - 1])
            phase_c_fin(BL - 1, states[BL - 1])

    nc.compile()
    return nc


@functools.lru_cache(maxsize=1)
def _built():
    return build()


def prep_in_maps(query, keys, key_padding_mask, Wa_w, Wa_b, Ua_w, Ua_b, Va_w, Va_b):
    query = np.asarray(query, np.float32)
    keys = np.asarray(keys, np.float32)
    mask = np.asarray(key_padding_mask)
    wq = (query @ np.asarray(Wa_w, np.float32).T + np.asarray(Wa_b, np.float32)
          + np.asarray(Ua_b, np.float32))                      # [B, M]
    ua_t = np.ascontiguousarray(np.asarray(Ua_w, np.float32).T).astype(BF16NP).reshape(ET, 128, M)
    vat = np.zeros((128, 8), BF16NP)
    vat[:, 0:MT] = np.asarray(Va_w, np.float32)[0].reshape(MT, 128).T.astype(BF16NP)
    maskb = (np.where(mask, NEG, np.float32(0.0)) + np.asarray(Va_b, np.float32)[0]).astype(np.float32)
    keys_bf = keys.astype(BF16NP)
    keys_t = np.ascontiguousarray(
        keys_bf.transpose(0, 2, 1).reshape(B, ET, 128, SC, 512).transpose(0, 3, 1, 2, 4)
    )  # [B, SC, ET, 128, 512]

    in_maps = []
    for c in range(N_CORES):
        sl = slice(c * BL, (c + 1) * BL)
        wq_c = np.concatenate(
            [
                wq[sl].reshape(BL, MT, 128).transpose(2, 0, 1).reshape(128, BL * MT),
                np.ones((128, 1), np.float32),
            ],
            axis=1,
        )
        in_maps.append({
            "keys_nat": np.ascontiguousarray(keys_bf[sl]).reshape(BL, ST, 128, E),
            "keys_t": np.ascontiguousarray(keys_t[sl]),
            "ua_t": ua_t,
            "vat": vat,
            "wq_b": np.ascontiguousarray(wq_c),
            "maskb": np.ascontiguousarray(maskb[sl]),
        })
    return in_maps


def run(in_maps, **kwargs):
    nc = _built()
    return run_bass_kernel_spmd(nc, in_maps, list(range(N_CORES)), **kwargs)


def kernel(**inputs):
    res = run(prep_in_maps(**inputs))
    out = np.stack([r["out_ctx"] for r in res.results])  # [N_CORES, BL, E]
    return out.reshape(B, 1, E)


# revision 29
# speedup vs baseline: 1.0232x; 1.0232x over previous
"""Bahdanau attention Trainium2 kernel.

reference:
    wq = query @ Wa_w.T + Wa_b                       # [B, M]
    uk = einsum('bse,me->bsm', keys, Ua_w) + Ua_b    # [B, S, M]
    e = tanh(wq[:, None, :] + uk)                    # [B, S, M]
    scores = einsum('bsm,om->bos', e, Va_w) + Va_b   # [B, 1, S]
    scores = where(mask, -inf, scores)
    attn = softmax(scores, axis=-1)
    context = einsum('bos,bse->boe', attn, keys)     # [B, 1, 2d]

Sharding: data-parallel over batch, 4 batches per core on 8 cores.
Matmul data path in bf16 (1 cycle/row on the PE; fp32r streams at ~2
cycles/row on this HW), accumulation in fp32 PSUM.
"""

import functools

import numpy as np
import ml_dtypes

import concourse.bacc as bacc
import concourse.tile as tile
from concourse import mybir
from concourse.bass_utils import run_bass_kernel_spmd

N_CORES = 8
B, S, D = 32, 2048, 512
E = 2 * D            # 1024 (key embedding dim)
M = D                # 512  (model dim)
BL = B // N_CORES    # 4 batches per core
SC = S // 512        # 4 s-chunks of 512
ST = S // 128        # 16 s-tiles of 128
ET = E // 128        # 8 e-tiles of 128
MT = M // 128        # 4 m-tiles of 128
NAT_G = 2            # s-tiles per natural-keys DMA
NEG = np.float32(-1e30)
BF16NP = ml_dtypes.bfloat16

F32 = mybir.dt.float32
BF16 = mybir.dt.bfloat16
AF = mybir.ActivationFunctionType


def build():
    nc = bacc.Bacc(None, target_bir_lowering=False)

    keys_nat = nc.dram_tensor("keys_nat", [BL, ST, 128, E], BF16, kind="ExternalInput")
    keys_t = nc.dram_tensor("keys_t", [BL, SC, ET, 128, 512], BF16, kind="ExternalInput")
    ua_t = nc.dram_tensor("ua_t", [ET, 128, M], BF16, kind="ExternalInput")
    vat = nc.dram_tensor("vat", [128, 8], BF16, kind="ExternalInput")
    wq_b = nc.dram_tensor("wq_b", [128, BL * MT + 1], F32, kind="ExternalInput")
    maskb = nc.dram_tensor("maskb", [BL, S], F32, kind="ExternalInput")
    out_ctx = nc.dram_tensor("out_ctx", [BL, E], F32, kind="ExternalOutput")

    with tile.TileContext(nc) as tc:
        with (
            tc.tile_pool(name="const", bufs=1) as constp,
            tc.tile_pool(name="kt", bufs=4) as ktp,
            tc.tile_pool(name="nat", bufs=8) as natp,
            tc.tile_pool(name="ep", bufs=6) as ep,
            tc.tile_pool(name="sm", bufs=2) as smp,
            tc.tile_pool(name="ukps", bufs=2, space="PSUM") as ukps,
            tc.tile_pool(name="scps", bufs=2, space="PSUM") as scps,
            tc.tile_pool(name="sctps", bufs=2, space="PSUM") as sctps,
            tc.tile_pool(name="ctxps", bufs=1, space="PSUM") as ctxps,
        ):
            ua_sb = constp.tile([128, ET, M], BF16)
            # per-e-tile slices so the first uk matmul starts after ~128KB
            for et_i in range(ET):
                nc.scalar.dma_start(ua_sb[:, et_i, :], ua_t[et_i])
            vat_sb = constp.tile([128, 8], BF16)
            nc.gpsimd.dma_start(vat_sb[:], vat[:])
            wq_sb = constp.tile([128, BL * MT + 1], F32)
            nc.gpsimd.dma_start(wq_sb[:], wq_b[:])

            def phase_a_chunk(b, sc_i, state):
                """emit one s-chunk of the scores computation"""
                if sc_i == 0:
                    state["scores_row"] = smp.tile([1, S], F32, tag="scores", name="scores_row")
                scores_row = state["scores_row"]
                kt_sb = ktp.tile([128, ET, 512], BF16, tag="kt")
                if b == 0 and sc_i == 0:
                    # split the very first load so the PE starts early
                    for et_i in range(ET):
                        nc.sync.dma_start(kt_sb[:, et_i, :], keys_t[b, sc_i, et_i])
                else:
                    nc.sync.dma_start(kt_sb[:], keys_t[b, sc_i].rearrange("a p j -> p a j"))
                mask_sb = smp.tile([1, 512], F32, tag="mask", bufs=3)
                nc.gpsimd.dma_start(mask_sb[:], maskb[b : b + 1, sc_i * 512 : (sc_i + 1) * 512])
                ps_sc = scps.tile([1, 512], F32, tag="ps_sc")
                e_tiles = []
                for mt_i in range(MT):
                    ps_uk = ukps.tile([128, 512], F32, tag="ps_uk")
                    for et_i in range(ET):
                        nc.tensor.matmul(
                            ps_uk[:],
                            ua_sb[:, et_i, mt_i * 128 : (mt_i + 1) * 128],
                            kt_sb[:, et_i, :],
                            start=(et_i == 0),
                            stop=(et_i == ET - 1),
                        )
                    e_sb = ep.tile([128, 512], BF16, tag="e")
                    nc.scalar.activation(
                        e_sb[:], ps_uk[:], AF.Tanh,
                        bias=wq_sb[:, b * MT + mt_i : b * MT + mt_i + 1],
                    )
                    e_tiles.append(e_sb)
                for mt_i in range(MT):
                    nc.tensor.matmul(
                        ps_sc[:],
                        vat_sb[:, mt_i : mt_i + 1],
                        e_tiles[mt_i][:],
                        start=(mt_i == 0),
                        stop=(mt_i == MT - 1),
                    )
                # scores_row = psum + mask(-1e30 or Va_b)
                nc.vector.tensor_add(
                    scores_row[0:1, sc_i * 512 : (sc_i + 1) * 512], ps_sc[:], mask_sb[:]
                )
                state.setdefault("sct_pending", []).append(sc_i)

            def flush_sct(state, upto):
                """PE transpose of finished score chunks: column j of ps_sct is
                scores[128j:128j+128].T @ [[1.0]] (K=1, M=128, N=1 fp32 matmul;
                fp32r/bf16 matmuls forbid odd N, plain fp32 allows it). Deferred
                one chunk so the PE never waits on the DVE mask-add."""
                if "ps_sct" not in state:
                    state["ps_sct"] = sctps.tile([128, ST], F32, tag="ps_sct", name="ps_sct")
                scores_row = state["scores_row"]
                pend = state["sct_pending"]
                while pend and (upto is None or pend[0] <= upto):
                    sc_i = pend.pop(0)
                    for j in range(sc_i * ST // SC, (sc_i + 1) * ST // SC):
                        nc.tensor.matmul(
                            state["ps_sct"][:, j : j + 1],
                            scores_row[0:1, j * 128 : (j + 1) * 128],
                            wq_sb[0:1, BL * MT : BL * MT + 1],
                            start=True,
                            stop=True,
                        )

            def phase_a_fin(b, state):
                """exp + denom + transposed exp"""
                scores_row = state["scores_row"]
                exp_row = smp.tile([1, S], F32, tag="exp_row")
                denom = smp.tile([1, 1], F32, tag="denom")
                nc.scalar.activation(exp_row[:], scores_row[:], AF.Exp, accum_out=denom[:])
                recip = smp.tile([1, 1], F32, tag="recip")
                nc.vector.reciprocal(recip[:], denom[:])
                flush_sct(state, None)
                ps_sct = state.pop("ps_sct")
                exp_t = smp.tile([128, ST], BF16, tag="exp_t")
                nc.scalar.activation(exp_t[:], ps_sct[:], AF.Exp)
                state["exp_t"] = exp_t
                state["recip"] = recip

            def phase_c_q(b, q, state):
                """emit one natural-keys group (NAT_G s-tiles) of the context matmul"""
                exp_t = state["exp_t"]
                if q == 0:
                    state["ps0"] = ctxps.tile([1, 512], F32, tag="ps_ctx0", name="ps0")
                    state["ps1"] = ctxps.tile([1, 512], F32, tag="ps_ctx1", name="ps1")
                ps0, ps1 = state["ps0"], state["ps1"]
                nat_sb = natp.tile([128, NAT_G, E], BF16, tag="nat")
                nc.scalar.dma_start(
                    nat_sb[:], keys_nat[b, q * NAT_G : (q + 1) * NAT_G].rearrange("t p e -> p t e")
                )
                for r in range(NAT_G):
                    st_i = q * NAT_G + r
                    nc.tensor.matmul(
                        ps0[:], exp_t[:, st_i : st_i + 1], nat_sb[:, r, 0:512],
                        start=(st_i == 0), stop=(st_i == ST - 1),
                    )
                    nc.tensor.matmul(
                        ps1[:], exp_t[:, st_i : st_i + 1], nat_sb[:, r, 512:1024],
                        start=(st_i == 0), stop=(st_i == ST - 1),
                    )

            def phase_c_fin(b, state):
                ctx_sb = smp.tile([1, E], F32, tag="ctx")
                nc.scalar.activation(ctx_sb[0:1, 0:512], state["ps0"][:], AF.Copy, scale=state["recip"][:])
                nc.scalar.activation(ctx_sb[0:1, 512:1024], state["ps1"][:], AF.Copy, scale=state["recip"][:])
                nc.sync.dma_start(out_ctx[b : b + 1, :], ctx_sb[:])

            # Software pipeline: interleave phase C of batch b with phase A of
            # batch b+1 at chunk granularity so PE and both DMA rings stay busy.
            NQ = ST // NAT_G
            states = [dict() for _ in range(BL)]
            for sc_i in range(SC):
                phase_a_chunk(0, sc_i, states[0])
                flush_sct(states[0], sc_i - 1)
            phase_a_fin(0, states[0])
            for b in range(1, BL):
                qn = 0
                for sc_i in range(SC):
                    phase_a_chunk(b, sc_i, states[b])
                    flush_sct(states[b], sc_i - 1)
                    while qn < (sc_i + 1) * NQ // SC:
                        phase_c_q(b - 1, qn, states[b - 1])
                        qn += 1
                phase_a_fin(b, states[b])
                phase_c_fin(b - 1, states[b - 1])
            for qn in range(NQ):
                phase_c_q(BL - 1, qn, states[BL - 1])
            phase_c_fin(BL - 1, states[BL - 1])

    nc.compile()
    return nc


@functools.lru_cache(maxsize=1)
def _built():
    return build()


def prep_in_maps(query, keys, key_padding_mask, Wa_w, Wa_b, Ua_w, Ua_b, Va_w, Va_b):
    query = np.asarray(query, np.float32)
    keys = np.asarray(keys, np.float32)
    mask = np.asarray(key_padding_mask)
    wq = (query @ np.asarray(Wa_w, np.float32).T + np.asarray(Wa_b, np.float32)
          + np.asarray(Ua_b, np.float32))                      # [B, M]
    ua_t = np.ascontiguousarray(np.asarray(Ua_w, np.float32).T).astype(BF16NP).reshape(ET, 128, M)
    vat = np.zeros((128, 8), BF16NP)
    vat[:, 0:MT] = np.asarray(Va_w, np.float32)[0].reshape(MT, 128).T.astype(BF16NP)
    maskb = (np.where(mask, NEG, np.float32(0.0)) + np.asarray(Va_b, np.float32)[0]).astype(np.float32)
    keys_bf = keys.astype(BF16NP)
    keys_t = np.ascontiguousarray(
        keys_bf.transpose(0, 2, 1).reshape(B, ET, 128, SC, 512).transpose(0, 3, 1, 2, 4)
    )  # [B, SC, ET, 128, 512]

    in_maps = []
    for c in range(N_CORES):
        sl = slice(c * BL, (c + 1) * BL)
        wq_c = np.concatenate(
            [
                wq[sl].reshape(BL, MT, 128).transpose(2, 0, 1).reshape(128, BL * MT),
                np.ones((128, 1), np.float32),
            ],
            axis=1,
        )
        in_maps.append({
            "keys_nat": np.ascontiguousarray(keys_bf[sl]).reshape(BL, ST, 128, E),
            "keys_t": np.ascontiguousarray(keys_t[sl]),
            "ua_t": ua_t,
            "vat": vat,
            "wq_b": np.ascontiguousarray(wq_c),
            "maskb": np.ascontiguousarray(maskb[sl]),
        })
    return in_maps


def run(in_maps, **kwargs):
    nc = _built()
    return run_bass_kernel_spmd(nc, in_maps, list(range(N_CORES)), **kwargs)


def kernel(**inputs):
    res = run(prep_in_maps(**inputs))
    out = np.stack([r["out_ctx"] for r in res.results])  # [N_CORES, BL, E]
    return out.reshape(B, 1, E)
